# revision 1
# baseline (speedup 1.0000x reference)
"""Trainium2 Bass kernel for nn_LocalConnectivity (diamond-ring circular stencil).

out[i,j] = sum_{d=1..5} w_d * sum_{|di|+|dj|=d} x[(i+di)%H, (j+dj)%W]

Strategy: row-shard across 8 NeuronCores (512 rows each + 5-row circular
halo, columns pre-padded with 5-col circular halo on host). Per core the
61-tap stencil is computed on the TensorEngine as 11 banded matmuls (one
per column shift dj in [-5,5]): PSUM[m, c] += W_dj[k, m] * strip[k, c+5+dj]
where W_dj is a [128, 118] constant band matrix holding the vertical taps
for that dj and the column shift rides the rhs access pattern for free.
float32r matmuls stream at 1 cycle/row (vs 4 for float32) at ~2e-4 rel err.
"""
import numpy as np
from contextlib import ExitStack

import concourse.bass as bass
import concourse.tile as tile
from concourse import bacc, mybir
from concourse.bass_utils import run_bass_kernel_spmd

N_CORES = 8
H = W = 4096
MAXD = 5
ROWS_PER_CORE = H // N_CORES          # 512
IN_ROWS = ROWS_PER_CORE + 2 * MAXD    # 522
IN_COLS = W + 2 * MAXD                # 4106
NCOL = 512                            # matmul free dim (one PSUM bank, fp32 max)
NCHUNK = W // NCOL                    # 8
M_OUT = 118                           # output rows per row-window (K=128 - 2*MAXD)
# row windows: (input_row_start, out_row_start, K, M)
WINDOWS = []
_o = 0
while _o < ROWS_PER_CORE:
    m = min(M_OUT, ROWS_PER_CORE - _o)
    WINDOWS.append((_o, _o, m + 2 * MAXD, m))
    _o += m

_CACHE = {}


def _band_weights(distance_weights: np.ndarray) -> np.ndarray:
    """w_flat [128, 11*118]: w_flat[k, (dj+5)*118 + m] = K2d[k-m-5, dj]."""
    wd = np.asarray(distance_weights, dtype=np.float32)
    w = np.zeros((11, 128, M_OUT), dtype=np.float32)
    for dj in range(-MAXD, MAXD + 1):
        for di in range(-MAXD, MAXD + 1):
            d = abs(di) + abs(dj)
            if not (1 <= d <= MAXD):
                continue
            m = np.arange(M_OUT)
            k = m + MAXD + di
            ok = (k >= 0) & (k < 128)
            w[dj + MAXD, k[ok], m[ok]] = wd[d - 1]
    return np.ascontiguousarray(w.transpose(1, 0, 2).reshape(128, 11 * M_OUT))


def _build():
    dtr = mybir.dt.float32r
    dtf = mybir.dt.float32
    nc = bacc.Bacc("TRN2", target_bir_lowering=False, debug=False,
                   num_devices=N_CORES)
    x = nc.dram_tensor("x", [IN_ROWS, IN_COLS], dtr, kind="ExternalInput").ap()
    wts = nc.dram_tensor("w", [128, 11 * M_OUT], dtr, kind="ExternalInput").ap()
    y = nc.dram_tensor("y", [ROWS_PER_CORE, W], dtf, kind="ExternalOutput").ap()

    with tile.TileContext(nc) as tc, ExitStack() as ctx:
        spool = ctx.enter_context(tc.tile_pool(name="strip", bufs=3))
        wpool = ctx.enter_context(tc.tile_pool(name="wts", bufs=1))
        opool = ctx.enter_context(tc.tile_pool(name="out", bufs=2))
        ppool = ctx.enter_context(tc.tile_pool(name="ps", bufs=8, space="PSUM"))

        CMID = IN_COLS // 2
        strips = []
        # Issue strip0 before the weights so the critical first window's
        # data transfer starts immediately; weights ride the idle sync queue.
        for wi, (in0, out0, kdim, m) in enumerate(WINDOWS):
            if wi == 0:
                st = spool.tile([128, IN_COLS], dtr, tag="strip")
                nc.gpsimd.dma_start(st[:kdim, :CMID], x[in0:in0 + kdim, :CMID])
                nc.scalar.dma_start(st[:kdim, CMID:], x[in0:in0 + kdim, CMID:])
                strips.append(st)
        wt = wpool.tile([128, 11 * M_OUT], dtr)
        nc.sync.dma_start(wt[:], wts[:])

        for wi, (in0, out0, kdim, m) in enumerate(WINDOWS):
            if wi == 0:
                st = strips[0]
            else:
                st = spool.tile([128, IN_COLS], dtr, tag="strip")
                nc.gpsimd.dma_start(st[:kdim, :CMID], x[in0:in0 + kdim, :CMID])
                nc.scalar.dma_start(st[:kdim, CMID:], x[in0:in0 + kdim, CMID:])
            ot = opool.tile([m, W], dtf, tag="out")
            for cc in range(NCHUNK):
                ps = ppool.tile([m, NCOL], dtf, tag="ps")
                for j, dj in enumerate(range(-MAXD, MAXD + 1)):
                    c0 = cc * NCOL + MAXD + dj
                    nc.tensor.matmul(
                        ps[:],
                        wt[:kdim, (dj + MAXD) * M_OUT:(dj + MAXD) * M_OUT + m],
                        st[:kdim, c0:c0 + NCOL],
                        start=(j == 0), stop=(j == 10),
                    )
                dst = ot[:, cc * NCOL:(cc + 1) * NCOL]
                if cc % 2 == 0:
                    nc.vector.tensor_copy(dst, ps[:])
                else:
                    nc.scalar.copy(dst, ps[:])
            # One fully-contiguous DRAM write per window (m full rows) so the
            # HW DGE fans it out across all 16 SDMA engines; keep stores off
            # the strip queues to avoid head-of-line blocking the prefetch.
            nc.sync.dma_start(y[out0:out0 + m, :], ot[:])
    nc.compile()
    return nc


def kernel(grid_spikes: np.ndarray, distance_weights: np.ndarray) -> np.ndarray:
    x = np.ascontiguousarray(grid_spikes, dtype=np.float32)
    assert x.shape == (H, W)
    if "nc" not in _CACHE:
        _CACHE["nc"] = _build()
    nc = _CACHE["nc"]

    w_flat = _band_weights(distance_weights)
    xpad = np.concatenate([x[:, -MAXD:], x, x[:, :MAXD]], axis=1)
    in_maps = []
    for c in range(N_CORES):
        rows = np.arange(c * ROWS_PER_CORE - MAXD,
                         c * ROWS_PER_CORE + ROWS_PER_CORE + MAXD) % H
        in_maps.append({"x": np.ascontiguousarray(xpad[rows]), "w": w_flat})

    res = run_bass_kernel_spmd(nc, in_maps, list(range(N_CORES)))
    out = np.concatenate([res.results[c]["y"] for c in range(N_CORES)], axis=0)
    return out.astype(np.float32)



# revision 2
# speedup vs baseline: 1.7103x; 1.7103x over previous
"""Trainium2 Bass kernel for nn_LocalConnectivity (diamond-ring circular stencil).

out[i,j] = sum_{d=1..5} w_d * sum_{|di|+|dj|=d} x[(i+di)%H, (j+dj)%W]

Strategy: row-shard across 8 NeuronCores (512 rows each + 5-row circular
halo, columns pre-padded with 5-col circular halo on host), all in bf16.
The 11x11 kernel K[di,dj] is symmetric in dj, so columns +dj and -dj share
one vertical band: DVE pre-sums the +/-dj shifted slices (5 tensor_adds per
128-row window) and the TensorEngine applies 6 banded matmuls per 512-col
chunk (vs 11 unpaired) accumulating in one PSUM bank. All DRAM<->SBUF DMAs
are column-split so every row is a separate strided descriptor run -- a
fully contiguous transfer becomes ONE run pinned to ONE 22.5 GB/s DMA
engine, while strided runs round-robin across all 16.
"""
import numpy as np
from contextlib import ExitStack

import concourse.bass as bass
import concourse.tile as tile
from concourse import bacc, mybir
from concourse.bass_utils import run_bass_kernel_spmd

try:
    import ml_dtypes
    _BF16 = np.dtype(ml_dtypes.bfloat16)
except ImportError:  # pragma: no cover
    _BF16 = None

N_CORES = 8
H = W = 4096
MAXD = 5
ROWS_PER_CORE = H // N_CORES          # 512
IN_ROWS = ROWS_PER_CORE + 2 * MAXD    # 522
IN_COLS = W + 2 * MAXD                # 4106
NCOL = 512                            # matmul free dim (one PSUM bank, fp32 max)
NCHUNK = W // NCOL                    # 8
M_OUT = 118                           # output rows per row-window (K=128 - 2*MAXD)
NBAND = MAXD + 1                      # 6 vertical bands (|dj| = 0..5)
# row windows: (input_row_start, out_row_start, K, M)
WINDOWS = []
_o = 0
while _o < ROWS_PER_CORE:
    m = min(M_OUT, ROWS_PER_CORE - _o)
    WINDOWS.append((_o, _o, m + 2 * MAXD, m))
    _o += m

_CACHE = {}


def _band_weights6(distance_weights: np.ndarray) -> np.ndarray:
    """w6 [128, 6*118]: w6[k, jj*118 + m] = K2d[k-m-5, jj] for |dj|=jj."""
    wd = np.asarray(distance_weights, dtype=np.float32)
    w = np.zeros((NBAND, 128, M_OUT), dtype=np.float32)
    for jj in range(NBAND):
        for di in range(-MAXD, MAXD + 1):
            d = abs(di) + jj
            if not (1 <= d <= MAXD):
                continue
            m = np.arange(M_OUT)
            k = m + MAXD + di
            ok = (k >= 0) & (k < 128)
            w[jj, k[ok], m[ok]] = wd[d - 1]
    w = np.ascontiguousarray(w.transpose(1, 0, 2).reshape(128, NBAND * M_OUT))
    return w.astype(_BF16)


def _build():
    dtb = mybir.dt.bfloat16
    nc = bacc.Bacc("TRN2", target_bir_lowering=False, debug=False,
                   num_devices=N_CORES)
    x = nc.dram_tensor("x", [IN_ROWS, IN_COLS], dtb, kind="ExternalInput").ap()
    wts = nc.dram_tensor("w", [128, NBAND * M_OUT], dtb,
                         kind="ExternalInput").ap()
    y = nc.dram_tensor("y", [ROWS_PER_CORE, W], dtb, kind="ExternalOutput").ap()

    with tile.TileContext(nc) as tc, ExitStack() as ctx:
        spool = ctx.enter_context(tc.tile_pool(name="strip", bufs=3))
        wpool = ctx.enter_context(tc.tile_pool(name="wts", bufs=1))
        mpool = ctx.enter_context(tc.tile_pool(name="sums", bufs=2))
        opool = ctx.enter_context(tc.tile_pool(name="out", bufs=2))
        ppool = ctx.enter_context(tc.tile_pool(name="ps", bufs=8, space="PSUM"))

        CMID = IN_COLS // 2
        strips = []

        def load_strip(in0, kdim):
            # Column-split halves: DRAM runs shorter than the row stride, so
            # each row is its own descriptor and fans across all DMA engines.
            st = spool.tile([128, IN_COLS], dtb, tag="strip")
            nc.gpsimd.dma_start(st[:kdim, :CMID], x[in0:in0 + kdim, :CMID])
            nc.scalar.dma_start(st[:kdim, CMID:], x[in0:in0 + kdim, CMID:])
            return st

        # Prefetch the first strip before the weights so the critical first
        # window's data is in flight immediately.
        strips.append(load_strip(WINDOWS[0][0], WINDOWS[0][2]))
        wt = wpool.tile([128, NBAND * M_OUT], dtb)
        WMID = (NBAND * M_OUT) // 2
        nc.sync.dma_start(wt[:, :WMID], wts[:, :WMID])
        nc.sync.dma_start(wt[:, WMID:], wts[:, WMID:])

        for wi, (in0, out0, kdim, m) in enumerate(WINDOWS):
            st = strips[wi] if wi == 0 else load_strip(in0, kdim)
            if wi == 0:
                strips.clear()  # drop python ref; pool owns lifetime

            # Paired column sums on DVE: s_jj[:, j] = x[:, j+5-jj] + x[:, j+5+jj]
            sm = mpool.tile([128, MAXD * W], dtb, tag="sums")
            for jj in range(1, MAXD + 1):
                nc.vector.tensor_add(
                    sm[:kdim, (jj - 1) * W:jj * W],
                    st[:kdim, MAXD - jj:MAXD - jj + W],
                    st[:kdim, MAXD + jj:MAXD + jj + W],
                )

            ot = opool.tile([m, W], dtb, tag="out")
            for cc in range(NCHUNK):
                ps = ppool.tile([m, NCOL], mybir.dt.float32, tag="ps")
                c0 = cc * NCOL
                nc.tensor.matmul(
                    ps[:], wt[:kdim, 0:m], st[:kdim, MAXD + c0:MAXD + c0 + NCOL],
                    start=True, stop=False,
                )
                for jj in range(1, MAXD + 1):
                    nc.tensor.matmul(
                        ps[:],
                        wt[:kdim, jj * M_OUT:jj * M_OUT + m],
                        sm[:kdim, (jj - 1) * W + c0:(jj - 1) * W + c0 + NCOL],
                        start=False, stop=(jj == MAXD),
                    )
                nc.scalar.copy(ot[:, c0:c0 + NCOL], ps[:])
            # Column-split stores (strided DRAM runs -> per-row descriptors
            # -> all 16 DMA engines). Two halves on the idle sync queue.
            HW2 = W // 2
            nc.sync.dma_start(y[out0:out0 + m, :HW2], ot[:, :HW2])
            nc.sync.dma_start(y[out0:out0 + m, HW2:], ot[:, HW2:])
    nc.compile()
    return nc


def _in_maps(grid_spikes: np.ndarray, distance_weights: np.ndarray):
    x = np.ascontiguousarray(grid_spikes, dtype=np.float32)
    assert x.shape == (H, W)
    w6 = _band_weights6(distance_weights)
    xpad = np.concatenate([x[:, -MAXD:], x, x[:, :MAXD]], axis=1).astype(_BF16)
    in_maps = []
    for c in range(N_CORES):
        rows = np.arange(c * ROWS_PER_CORE - MAXD,
                         c * ROWS_PER_CORE + ROWS_PER_CORE + MAXD) % H
        in_maps.append({"x": np.ascontiguousarray(xpad[rows]), "w": w6})
    return in_maps


def kernel(grid_spikes: np.ndarray, distance_weights: np.ndarray) -> np.ndarray:
    if "nc" not in _CACHE:
        _CACHE["nc"] = _build()
    nc = _CACHE["nc"]
    in_maps = _in_maps(grid_spikes, distance_weights)
    res = run_bass_kernel_spmd(nc, in_maps, list(range(N_CORES)))
    out = np.concatenate(
        [np.asarray(res.results[c]["y"]) for c in range(N_CORES)], axis=0)
    return out.astype(np.float32)


# revision 6
# speedup vs baseline: 1.8590x; 1.0869x over previous
"""Trainium2 Bass kernel for nn_LocalConnectivity (diamond-ring circular stencil).

out[i,j] = sum_{d=1..5} w_d * sum_{|di|+|dj|=d} x[(i+di)%H, (j+dj)%W]

Strategy: row-shard across 8 NeuronCores (512 rows each + 5-row circular
halo, columns pre-padded with 5-col circular halo on host), all in bf16.
The 11x11 kernel K[di,dj] is symmetric in dj, so columns +dj and -dj share
one vertical band: the +-dj shifted slices are pre-summed (jj=1..4 on DVE,
jj=5 on GpSimd to balance engine load) and the TensorEngine applies 6
banded matmuls per 512-col chunk accumulating in one PSUM bank. Pair-sums
are emitted in 1024-col blocks so matmuls for chunk c overlap sums for
c+2. All strips prefetch up front (bufs=5). DRAM-side APs of every DMA
are strided (column-split halves) so each row is its own descriptor run --
a fully contiguous transfer becomes ONE run pinned to ONE ~22 GB/s DMA
engine, while strided runs round-robin across all 16.
"""
import numpy as np
from contextlib import ExitStack

import concourse.bass as bass
import concourse.tile as tile
from concourse import bacc, mybir
from concourse.bass_utils import run_bass_kernel_spmd

try:
    import ml_dtypes
    _BF16 = np.dtype(ml_dtypes.bfloat16)
except ImportError:  # pragma: no cover
    _BF16 = None

N_CORES = 8
H = W = 4096
MAXD = 5
ROWS_PER_CORE = H // N_CORES          # 512
IN_ROWS = ROWS_PER_CORE + 2 * MAXD    # 522
IN_COLS = W + 2 * MAXD                # 4106
NCOL = 512                            # matmul free dim (one PSUM bank, fp32 max)
NCHUNK = W // NCOL                    # 8
M_OUT = 118                           # output rows per row-window (K=128 - 2*MAXD)
NBAND = MAXD + 1                      # 6 vertical bands (|dj| = 0..5)
NPAIR_DVE = 4                         # paired offsets computed on DVE (jj=1..4)
BLK = 1024                            # pair-sum emission granularity (cols)
# row windows: (input_row_start, out_row_start, K, M)
WINDOWS = []
_o = 0
while _o < ROWS_PER_CORE:
    m = min(M_OUT, ROWS_PER_CORE - _o)
    WINDOWS.append((_o, _o, m + 2 * MAXD, m))
    _o += m

_CACHE = {}


def _band_weights6(distance_weights: np.ndarray) -> np.ndarray:
    """w6 [128, 6*118]: w6[k, jj*118 + m] = K2d[k-m-5, jj] for |dj|=jj."""
    wd = np.asarray(distance_weights, dtype=np.float32)
    w = np.zeros((NBAND, 128, M_OUT), dtype=np.float32)
    for jj in range(NBAND):  # |dj| = 0..5
        for di in range(-MAXD, MAXD + 1):
            d = abs(di) + jj
            if not (1 <= d <= MAXD):
                continue
            m = np.arange(M_OUT)
            k = m + MAXD + di
            ok = (k >= 0) & (k < 128)
            w[jj, k[ok], m[ok]] = wd[d - 1]
    w = np.ascontiguousarray(w.transpose(1, 0, 2).reshape(128, NBAND * M_OUT))
    return w.astype(_BF16)


def _build():
    dtb = mybir.dt.bfloat16
    nc = bacc.Bacc("TRN2", target_bir_lowering=False, debug=False,
                   num_devices=N_CORES)
    x = nc.dram_tensor("x", [IN_ROWS, IN_COLS], dtb, kind="ExternalInput").ap()
    wts = nc.dram_tensor("w", [128, NBAND * M_OUT], dtb,
                         kind="ExternalInput").ap()
    y = nc.dram_tensor("y", [ROWS_PER_CORE, W], dtb, kind="ExternalOutput").ap()

    with tile.TileContext(nc) as tc, ExitStack() as ctx:
        spool = ctx.enter_context(tc.tile_pool(name="strip", bufs=5))
        wpool = ctx.enter_context(tc.tile_pool(name="wts", bufs=1))
        mpool = ctx.enter_context(tc.tile_pool(name="sums", bufs=2))
        gpool = ctx.enter_context(tc.tile_pool(name="sum5", bufs=2))
        opool = ctx.enter_context(tc.tile_pool(name="out", bufs=2))
        ppool = ctx.enter_context(tc.tile_pool(name="ps", bufs=8, space="PSUM"))

        CMID = IN_COLS // 2
        strips = []

        def load_strip(in0, kdim):
            # Column-split halves: DRAM runs shorter than the row stride, so
            # each row is its own descriptor and fans across all DMA engines.
            st = spool.tile([128, IN_COLS], dtb, tag="strip")
            nc.gpsimd.dma_start(st[:kdim, :CMID], x[in0:in0 + kdim, :CMID])
            nc.scalar.dma_start(st[:kdim, CMID:], x[in0:in0 + kdim, CMID:])
            return st

        # Prefetch ALL strips up front (they fit in SBUF); first strip before
        # the weights so the critical first window's data is in flight first.
        strips.append(load_strip(WINDOWS[0][0], WINDOWS[0][2]))
        wt = wpool.tile([128, NBAND * M_OUT], dtb)
        WMID = (NBAND * M_OUT) // 2
        nc.sync.dma_start(wt[:, :WMID], wts[:, :WMID])
        nc.sync.dma_start(wt[:, WMID:], wts[:, WMID:])
        for (in0, _o0, kdim, _m) in WINDOWS[1:]:
            strips.append(load_strip(in0, kdim))

        for wi, (in0, out0, kdim, m) in enumerate(WINDOWS):
            st = strips[wi]

            # Paired column sums, emitted per 1024-col block so the matmuls
            # for chunk c can start while block c//2+1's sums compute:
            # s_jj[:, j] = x[:, j+5-jj] + x[:, j+5+jj]
            sm = mpool.tile([128, NPAIR_DVE * W], dtb, tag="sums")
            s5 = gpool.tile([128, W], dtb, tag="sum5")
            for b0 in range(0, W, BLK):
                for jj in range(1, NPAIR_DVE + 1):
                    nc.vector.tensor_add(
                        sm[:kdim, (jj - 1) * W + b0:(jj - 1) * W + b0 + BLK],
                        st[:kdim, MAXD - jj + b0:MAXD - jj + b0 + BLK],
                        st[:kdim, MAXD + jj + b0:MAXD + jj + b0 + BLK],
                    )
                nc.gpsimd.tensor_add(
                    s5[:kdim, b0:b0 + BLK],
                    st[:kdim, b0:b0 + BLK],
                    st[:kdim, 2 * MAXD + b0:2 * MAXD + b0 + BLK],
                )

            ot = opool.tile([m, W], dtb, tag="out")
            for cc in range(NCHUNK):
                ps = ppool.tile([m, NCOL], mybir.dt.float32, tag="ps")
                c0 = cc * NCOL
                nc.tensor.matmul(
                    ps[:], wt[:kdim, 0:m], st[:kdim, MAXD + c0:MAXD + c0 + NCOL],
                    start=True, stop=False,
                )
                for jj in range(1, NPAIR_DVE + 1):
                    nc.tensor.matmul(
                        ps[:],
                        wt[:kdim, jj * M_OUT:jj * M_OUT + m],
                        sm[:kdim, (jj - 1) * W + c0:(jj - 1) * W + c0 + NCOL],
                        start=False, stop=False,
                    )
                nc.tensor.matmul(
                    ps[:],
                    wt[:kdim, MAXD * M_OUT:MAXD * M_OUT + m],
                    s5[:kdim, c0:c0 + NCOL],
                    start=False, stop=True,
                )
                nc.scalar.copy(ot[:, c0:c0 + NCOL], ps[:])
                # Store each half-window as soon as its drains land; halves
                # split again column-wise across two queues so DRAM runs stay
                # strided (per-row descriptors -> all 16 DMA engines).
                if cc == NCHUNK // 2 - 1 or cc == NCHUNK - 1:
                    h0 = 0 if cc == NCHUNK // 2 - 1 else W // 2
                    q = W // 4
                    nc.sync.dma_start(y[out0:out0 + m, h0:h0 + q],
                                      ot[:, h0:h0 + q])
                    nc.gpsimd.dma_start(y[out0:out0 + m, h0 + q:h0 + 2 * q],
                                        ot[:, h0 + q:h0 + 2 * q])
    nc.compile()
    return nc


def _in_maps(grid_spikes: np.ndarray, distance_weights: np.ndarray):
    x = np.ascontiguousarray(grid_spikes, dtype=np.float32)
    assert x.shape == (H, W)
    w6 = _band_weights6(distance_weights)
    xpad = np.concatenate([x[:, -MAXD:], x, x[:, :MAXD]], axis=1).astype(_BF16)
    in_maps = []
    for c in range(N_CORES):
        rows = np.arange(c * ROWS_PER_CORE - MAXD,
                         c * ROWS_PER_CORE + ROWS_PER_CORE + MAXD) % H
        in_maps.append({"x": np.ascontiguousarray(xpad[rows]), "w": w6})
    return in_maps


def kernel(grid_spikes: np.ndarray, distance_weights: np.ndarray) -> np.ndarray:
    if "nc" not in _CACHE:
        _CACHE["nc"] = _build()
    nc = _CACHE["nc"]
    in_maps = _in_maps(grid_spikes, distance_weights)
    res = run_bass_kernel_spmd(nc, in_maps, list(range(N_CORES)))
    out = np.concatenate(
        [np.asarray(res.results[c]["y"]) for c in range(N_CORES)], axis=0)
    return out.astype(np.float32)


# revision 11
# speedup vs baseline: 2.2647x; 1.2182x over previous
"""Trainium2 Bass kernel for nn_LocalConnectivity (diamond-ring circular stencil).

out[i,j] = sum_{d=1..5} w_d * sum_{|di|+|dj|=d} x[(i+di)%H, (j+dj)%W]

Strategy: row-shard across 8 NeuronCores (512 rows each + 5-row circular
halo, columns pre-padded with 5-col circular halo on host), all in bf16.
The 11x11 kernel K[di,dj] is symmetric in dj, so columns +dj and -dj share
one vertical band: DVE pre-sums the +-dj shifted slices (jj-major, 1024-col
blocks) and the TensorEngine applies 6 banded matmuls per 512-col chunk,
jj-major across all 8 PSUM banks so consecutive LDWEIGHTS reuse the same
stationary band. All strips prefetch up front (bufs=5). DRAM-side APs of
every DMA are strided (column-split) so each row is its own descriptor
run -- a fully contiguous transfer becomes ONE run pinned to ONE ~22 GB/s
DMA engine, while strided runs round-robin across all 16.
"""
import numpy as np
from contextlib import ExitStack

import concourse.bass as bass
import concourse.tile as tile
from concourse import bacc, mybir
from concourse.bass_utils import run_bass_kernel_spmd

try:
    import ml_dtypes
    _BF16 = np.dtype(ml_dtypes.bfloat16)
except ImportError:  # pragma: no cover
    _BF16 = None

N_CORES = 8
H = W = 4096
MAXD = 5
ROWS_PER_CORE = H // N_CORES          # 512
IN_ROWS = ROWS_PER_CORE + 2 * MAXD    # 522
IN_COLS = W + 2 * MAXD                # 4106
NCOL = 512                            # matmul free dim (one PSUM bank, fp32 max)
NCHUNK = W // NCOL                    # 8
M_OUT = 118                           # output rows per row-window (K=128 - 2*MAXD)
NBAND = MAXD + 1                      # 6 vertical bands (|dj| = 0..5)
BLK = 1024                            # pair-sum emission granularity (cols)
# row windows: (input_row_start, out_row_start, K, M)
WINDOWS = []
_o = 0
while _o < ROWS_PER_CORE:
    m = min(M_OUT, ROWS_PER_CORE - _o)
    WINDOWS.append((_o, _o, m + 2 * MAXD, m))
    _o += m

_CACHE = {}


def _band_weights6(distance_weights: np.ndarray) -> np.ndarray:
    """w6 [128, 6*118]: w6[k, jj*118 + m] = K2d[k-m-5, jj] for |dj|=jj."""
    wd = np.asarray(distance_weights, dtype=np.float32)
    w = np.zeros((NBAND, 128, M_OUT), dtype=np.float32)
    for jj in range(NBAND):  # |dj| = 0..5
        for di in range(-MAXD, MAXD + 1):
            d = abs(di) + jj
            if not (1 <= d <= MAXD):
                continue
            m = np.arange(M_OUT)
            k = m + MAXD + di
            ok = (k >= 0) & (k < 128)
            w[jj, k[ok], m[ok]] = wd[d - 1]
    w = np.ascontiguousarray(w.transpose(1, 0, 2).reshape(128, NBAND * M_OUT))
    return w.astype(_BF16)


def _build():
    dtb = mybir.dt.bfloat16
    nc = bacc.Bacc("TRN2", target_bir_lowering=False, debug=False,
                   num_devices=N_CORES)
    x = nc.dram_tensor("x", [IN_ROWS, IN_COLS], dtb, kind="ExternalInput").ap()
    wts = nc.dram_tensor("w", [128, NBAND * M_OUT], dtb,
                         kind="ExternalInput").ap()
    y = nc.dram_tensor("y", [ROWS_PER_CORE, W], dtb, kind="ExternalOutput").ap()

    with tile.TileContext(nc) as tc, ExitStack() as ctx:
        spool = ctx.enter_context(tc.tile_pool(name="strip", bufs=5))
        wpool = ctx.enter_context(tc.tile_pool(name="wts", bufs=1))
        mpool = ctx.enter_context(tc.tile_pool(name="sums", bufs=2))
        opool = ctx.enter_context(tc.tile_pool(name="out", bufs=2))
        ppool = ctx.enter_context(tc.tile_pool(name="ps", bufs=8, space="PSUM"))

        CMID = IN_COLS // 2
        strips = []

        def load_strip(in0, kdim):
            # Column-split halves: DRAM runs shorter than the row stride, so
            # each row is its own descriptor and fans across all DMA engines.
            st = spool.tile([128, IN_COLS], dtb, tag="strip")
            nc.gpsimd.dma_start(st[:kdim, :CMID], x[in0:in0 + kdim, :CMID])
            nc.scalar.dma_start(st[:kdim, CMID:], x[in0:in0 + kdim, CMID:])
            return st

        # Prefetch ALL strips up front (they fit in SBUF); first strip before
        # the weights so the critical first window's data is in flight first.
        strips.append(load_strip(WINDOWS[0][0], WINDOWS[0][2]))
        wt = wpool.tile([128, NBAND * M_OUT], dtb)
        WMID = (NBAND * M_OUT) // 2
        nc.sync.dma_start(wt[:, :WMID], wts[:, :WMID])
        nc.sync.dma_start(wt[:, WMID:], wts[:, WMID:])
        for (in0, _o0, kdim, _m) in WINDOWS[1:]:
            strips.append(load_strip(in0, kdim))

        for wi, (in0, out0, kdim, m) in enumerate(WINDOWS):
            st = strips[wi]

            # Paired column sums on DVE, jj-major in 1024-col blocks so each
            # band's sum stream stays ahead of the PE's same-band pass:
            # s_jj[:, j] = x[:, j+5-jj] + x[:, j+5+jj]
            sm = mpool.tile([128, MAXD * W], dtb, tag="sums")
            for jj in range(1, MAXD + 1):
                for b0 in range(0, W, BLK):
                    nc.vector.tensor_add(
                        sm[:kdim, (jj - 1) * W + b0:(jj - 1) * W + b0 + BLK],
                        st[:kdim, MAXD - jj + b0:MAXD - jj + b0 + BLK],
                        st[:kdim, MAXD + jj + b0:MAXD + jj + b0 + BLK],
                    )

            # jj-major matmul passes: one stationary band applied to all 8
            # chunks (one PSUM bank each) before switching bands, so
            # consecutive LDWEIGHTS hit the same stationary.
            ot = opool.tile([m, W], dtb, tag="out")
            pss = [ppool.tile([m, NCOL], mybir.dt.float32, tag="ps",
                              name=f"ps_w{wi}c{cc}")
                   for cc in range(NCHUNK)]
            for jj in range(NBAND):
                for cc in range(NCHUNK):
                    c0 = cc * NCOL
                    if jj == 0:
                        rhs = st[:kdim, MAXD + c0:MAXD + c0 + NCOL]
                    else:
                        rhs = sm[:kdim, (jj - 1) * W + c0:(jj - 1) * W + c0 + NCOL]
                    nc.tensor.matmul(
                        pss[cc], wt[:kdim, jj * M_OUT:jj * M_OUT + m], rhs,
                        start=(jj == 0), stop=(jj == NBAND - 1),
                    )
            for cc in range(NCHUNK):
                c0 = cc * NCOL
                nc.scalar.copy(ot[:, c0:c0 + NCOL], pss[cc])
                # Store each half-window as soon as its drains land; halves
                # split again column-wise across two queues so DRAM runs stay
                # strided (per-row descriptors -> all 16 DMA engines).
                if cc == NCHUNK // 2 - 1 or cc == NCHUNK - 1:
                    h0 = 0 if cc == NCHUNK // 2 - 1 else W // 2
                    q = W // 4
                    nc.sync.dma_start(y[out0:out0 + m, h0:h0 + q],
                                      ot[:, h0:h0 + q])
                    nc.gpsimd.dma_start(y[out0:out0 + m, h0 + q:h0 + 2 * q],
                                        ot[:, h0 + q:h0 + 2 * q])
    nc.compile()
    return nc


def _in_maps(grid_spikes: np.ndarray, distance_weights: np.ndarray):
    x = np.ascontiguousarray(grid_spikes, dtype=np.float32)
    assert x.shape == (H, W)
    w6 = _band_weights6(distance_weights)
    xpad = np.concatenate([x[:, -MAXD:], x, x[:, :MAXD]], axis=1).astype(_BF16)
    in_maps = []
    for c in range(N_CORES):
        rows = np.arange(c * ROWS_PER_CORE - MAXD,
                         c * ROWS_PER_CORE + ROWS_PER_CORE + MAXD) % H
        in_maps.append({"x": np.ascontiguousarray(xpad[rows]), "w": w6})
    return in_maps


def kernel(grid_spikes: np.ndarray, distance_weights: np.ndarray) -> np.ndarray:
    if "nc" not in _CACHE:
        _CACHE["nc"] = _build()
    nc = _CACHE["nc"]
    in_maps = _in_maps(grid_spikes, distance_weights)
    res = run_bass_kernel_spmd(nc, in_maps, list(range(N_CORES)))
    out = np.concatenate(
        [np.asarray(res.results[c]["y"]) for c in range(N_CORES)], axis=0)
    return out.astype(np.float32)
